# revision 1
# baseline (speedup 1.0000x reference)
"""MoE top-2 routed FFN (B=4, S=2048, D=1024, H=2048, E=8) on 8 TRN2 NeuronCores.

Strategy (expert-parallel, matching the sharding hint):
  - Host computes the tiny gate (softmax top-2) and builds per-expert token
    lists ("all-to-all dispatch" done at the sharding step).
  - Core e receives the tokens routed to expert e (gathered, transposed,
    zero-padded to capacity C), plus expert e's weights pre-packed into the
    exact tile layouts the kernel consumes.
  - Each core runs a dense FFN  out = coef * ((relu(x@W1.T)^2 * (x@W3.T)) @ W2.T)
    over its C tokens.  All matmuls run in bf16 with fp32 PSUM accumulation
    (measured end-to-end rel err ~4e-3); coefficients and outputs stay fp32.
  - Host scatter-adds the per-expert outputs back ("combine").

Per-core kernel structure (single pass, weights read once):
  phase 1: for each of 16 H-tiles m: psA = W1m @ xT, psB = W3m @ xT (PSUM),
           gT[m] = relu(psA)^2 * psB  (DVE, bf16)   [H, C] layout
  phase 2: for each 128-token tile: out[tok, :] = (gT.T @ W2T) * coef  (PSUM->DVE->DRAM)
"""

import os
import sys

import numpy as np

if os.path.isdir("/opt/trn_rl_repo") and "/opt/trn_rl_repo" not in sys.path:
    sys.path.insert(0, "/opt/trn_rl_repo")

import ml_dtypes

import concourse.bacc as bacc
import concourse.mybir as mybir
from concourse.bass_utils import run_bass_kernel_spmd
from concourse.tile import TileContext

B, S, D, H, E = 4, 2048, 1024, 2048, 8
N = B * S
P = 128
KT = D // P   # 8 contraction tiles over D
MT = H // P   # 16 tiles over H

F32 = mybir.dt.float32
BF16 = mybir.dt.bfloat16
BF16_NP = ml_dtypes.bfloat16

# Set by test harness to capture profiling info.
TRACE = False
LAST_RESULTS = None


def _token_groups(c0, cw):
    """Split [c0, c0+cw) into moving-dim groups of at most 512."""
    groups = []
    rem = cw
    off = c0
    while rem > 0:
        if 512 < rem < 768:
            g = max(min(rem - 256, 512), 256)
        else:
            g = min(512, rem)
        groups.append((off, g))
        off += g
        rem -= g
    return groups


def build_kernel(C):
    TT = C // P
    nc = bacc.Bacc("TRN2", target_bir_lowering=False)

    xt = nc.dram_tensor("xt", [KT, P, C], BF16, kind="ExternalInput")
    w1p = nc.dram_tensor("w1p", [MT, P, KT * P], BF16, kind="ExternalInput")
    w3p = nc.dram_tensor("w3p", [MT, P, KT * P], BF16, kind="ExternalInput")
    w2p = nc.dram_tensor("w2p", [MT, P, D], BF16, kind="ExternalInput")
    cf = nc.dram_tensor("cf", [P, TT], F32, kind="ExternalInput")
    out = nc.dram_tensor("out", [TT, 2, P, 512], F32, kind="ExternalOutput")

    with TileContext(nc) as tc:
        with (
            tc.tile_pool(name="xt_pool", bufs=KT) as xt_pool,
            tc.tile_pool(name="g_pool", bufs=1) as g_pool,
            tc.tile_pool(name="w13_pool", bufs=2) as w13_pool,
            tc.tile_pool(name="w2_pool", bufs=MT) as w2_pool,
            tc.tile_pool(name="tmp_pool", bufs=2) as tmp_pool,
            tc.tile_pool(name="ob_pool", bufs=3) as ob_pool,
            tc.tile_pool(name="const_pool", bufs=1) as const_pool,
            tc.tile_pool(name="psAB", bufs=3, space="PSUM") as psAB_pool,
            tc.tile_pool(name="psO", bufs=2, space="PSUM") as psO_pool,
        ):
            # --- PE warmup: flip the HAM clock gate (1.2->2.4GHz) before the
            # first real matmul's data lands. Depends only on a local memset,
            # so it starts as soon as the PE sequencer is live. ---------------
            warm = const_pool.tile([P, 512], BF16, tag="warm")
            nc.any.memset(warm[:], 0.0)
            pswarm = psO_pool.tile([P, 512], F32, tag="psO", name="pswarm")
            for i in range(9):
                nc.tensor.matmul(pswarm[:], warm[:, :P], warm[:],
                                 start=(i == 0), stop=(i == 8))
            warmsink = const_pool.tile([P, 1], F32, tag="warmsink")
            nc.vector.tensor_scalar_mul(warmsink[:], pswarm[:, :1], 0.0)

            # XT split into head (first token group, unblocks PE fast) + tail
            HEAD = min(512, C)
            xts_head, xts_tail = [], []

            def xt_slice(k, g0, gw):
                if g0 < HEAD:
                    assert g0 + gw <= HEAD
                    return xts_head[k][:, g0:g0 + gw]
                return xts_tail[k][:, g0 - HEAD:g0 - HEAD + gw]

            w2ts = []

            # --- phase 1: gT[h, tok] = relu(W1 @ xT)^2 * (W3 @ xT) ----------
            gts = []
            for m in range(MT):
                gt = g_pool.tile([P, C], BF16, tag=f"g{m}", name=f"g_{m}")
                gts.append(gt)

            def do_groups(m, w1t, w3t, pr):
                # pr: 1-2 token groups done with a shared k loop, so
                # consecutive matmuls share the stationary weight slice and
                # bass emits one LDWEIGHTS per k instead of one per matmul.
                psAs, psBs = [], []
                for g0, gw in pr:
                    psAs.append(psAB_pool.tile([P, 512], F32, tag="psA",
                                               name=f"psA_{m}_{g0}"))
                    psBs.append(psAB_pool.tile([P, 512], F32, tag="psB",
                                               name=f"psB_{m}_{g0}"))
                for k in range(KT):
                    for (g0, gw), ps in zip(pr, psAs):
                        nc.tensor.matmul(
                            ps[:, :gw],
                            w1t[:, k * P:(k + 1) * P],
                            xt_slice(k, g0, gw),
                            start=(k == 0),
                            stop=(k == KT - 1),
                        )
                for k in range(KT):
                    for (g0, gw), ps in zip(pr, psBs):
                        nc.tensor.matmul(
                            ps[:, :gw],
                            w3t[:, k * P:(k + 1) * P],
                            xt_slice(k, g0, gw),
                            start=(k == 0),
                            stop=(k == KT - 1),
                        )
                for (g0, gw), psA, psB in zip(pr, psAs, psBs):
                    r = tmp_pool.tile([P, 512], F32, tag="r",
                                      name=f"r_{m}_{g0}")
                    nc.vector.tensor_relu(r[:, :gw], psA[:, :gw])
                    t2 = tmp_pool.tile([P, 512], F32, tag="t2",
                                       name=f"t2_{m}_{g0}")
                    nc.vector.tensor_mul(t2[:, :gw], r[:, :gw], r[:, :gw])
                    nc.vector.tensor_mul(
                        gts[m][:, g0:g0 + gw],
                        t2[:, :gw],
                        psB[:, :gw],
                    )

            for m in range(MT):
                w1t = w13_pool.tile([P, KT * P], BF16, tag="w1t",
                                    name=f"w1_{m}")
                nc.sync.dma_start(w1t[:], w1p[m])
                w3t = w13_pool.tile([P, KT * P], BF16, tag="w3t",
                                    name=f"w3_{m}")
                nc.sync.dma_start(w3t[:], w3p[m])
                if m == 0:
                    # DMA order: w1[m0], w3[m0] -> x heads -> x tails.
                    # The first psA matmuls start as soon as w1[m0]+xh land;
                    # everything later streams under compute.
                    for k in range(KT):
                        xh = xt_pool.tile([P, HEAD], BF16, tag="xh",
                                          name=f"xth_{k}")
                        nc.sync.dma_start(xh[:], xt[k][:, :HEAD])
                        xts_head.append(xh)
                    if C > HEAD:
                        for k in range(KT):
                            xtl = xt_pool.tile([P, C - HEAD], BF16, tag="xl",
                                               name=f"xtt_{k}")
                            nc.sync.dma_start(xtl[:], xt[k][:, HEAD:])
                            xts_tail.append(xtl)
                for g in _token_groups(0, C):
                    do_groups(m, w1t, w3t, [g])

            # W2 + coef loads emitted after phase-1 DMAs: they ride the idle
            # DMA tail of phase 1, well before phase 2 needs them, without
            # delaying PE start.
            cft = const_pool.tile([P, TT], F32, tag="cft")
            nc.sync.dma_start(cft[:], cf[:])
            for hk in range(MT):
                w2t = w2_pool.tile([P, D], BF16, tag="w2t", name=f"w2_{hk}")
                nc.sync.dma_start(w2t[:], w2p[hk])
                w2ts.append(w2t)

            # --- phase 2: out[tok, d] = coef * (g.T @ W2T) ------------------
            for t in range(TT):
                for dg in range(2):
                    if t == TT - 1 and dg == 1:
                        # split the final accumulation into quarters so each
                        # piece's scale+store overlaps the PE's remaining
                        # matmuls, shortening the post-PE tail chain
                        for h in range(4):
                            pso = psO_pool.tile([P, 512], F32, tag="psO",
                                                name=f"psO_{t}_{dg}_{h}")
                            for hk in range(MT):
                                nc.tensor.matmul(
                                    pso[:, :128],
                                    gts[hk][:, t * P:(t + 1) * P],
                                    w2ts[hk][:, dg * 512 + h * 128:
                                             dg * 512 + (h + 1) * 128],
                                    start=(hk == 0),
                                    stop=(hk == MT - 1),
                                )
                            ob = ob_pool.tile([P, 512], F32, tag="ob",
                                              name=f"ob_{t}_{dg}_{h}")
                            nc.vector.tensor_scalar_mul(ob[:, :128],
                                                        pso[:, :128],
                                                        cft[:, t:t + 1])
                            nc.sync.dma_start(
                                out[t, dg][:, h * 128:(h + 1) * 128],
                                ob[:, :128])
                        continue
                    pso = psO_pool.tile([P, 512], F32, tag="psO",
                                        name=f"psO_{t}_{dg}")
                    for hk in range(MT):
                        nc.tensor.matmul(
                            pso[:],
                            gts[hk][:, t * P:(t + 1) * P],
                            w2ts[hk][:, dg * 512:(dg + 1) * 512],
                            start=(hk == 0),
                            stop=(hk == MT - 1),
                        )
                    ob = ob_pool.tile([P, 512], F32, tag="ob",
                                      name=f"ob_{t}_{dg}")
                    nc.vector.tensor_scalar_mul(ob[:], pso[:],
                                                cft[:, t:t + 1])
                    nc.sync.dma_start(out[t, dg], ob[:])

    if not nc.is_finalized():
        nc.finalize()
    return nc


def kernel(x, W1, W2, W3, gate_w, gate_b):
    global LAST_RESULTS

    xf = np.ascontiguousarray(x.reshape(N, D).astype(np.float32, copy=False))

    # ---- gate: softmax + top-2 (tiny, done on host) ------------------------
    logits = xf @ gate_w.T.astype(np.float32) + gate_b.astype(np.float32)
    logits -= logits.max(axis=-1, keepdims=True)
    probs = np.exp(logits)
    probs /= probs.sum(axis=-1, keepdims=True)
    order = np.argsort(-probs, axis=-1, kind="stable")
    i1, i2 = order[:, 0], order[:, 1]
    ar = np.arange(N)
    p1, p2 = probs[ar, i1], probs[ar, i2]
    ps = p1 + p2
    c1, c2 = p1 / ps, p2 / ps

    idx_list, coef_list = [], []
    for e in range(E):
        m1 = i1 == e
        m2 = i2 == e
        ide = np.nonzero(m1 | m2)[0]
        ce = np.where(m1[ide], c1[ide], c2[ide]).astype(np.float32)
        idx_list.append(ide)
        coef_list.append(ce)

    nmax = max(len(i) for i in idx_list)
    C = max(((nmax + P - 1) // P) * P, 512)
    TT = C // P

    # ---- per-core input packing -------------------------------------------
    in_maps = []
    for e in range(E):
        ide, ce = idx_list[e], coef_list[e]
        ne = len(ide)

        xg = np.zeros((C, D), np.float32)
        xg[:ne] = xf[ide]
        xt_np = np.ascontiguousarray(xg.T).reshape(KT, P, C).astype(BF16_NP)

        w1e = np.asarray(W1[e], np.float32)  # [H, D]
        w3e = np.asarray(W3[e], np.float32)  # [H, D]
        w2e = np.asarray(W2[e], np.float32)  # [D, H]
        # [m, h, k, d] -> [m, d, k, h] : packed[m][d, k*128+h] = W1[m*128+h, k*128+d]
        w1p_np = np.ascontiguousarray(
            w1e.reshape(MT, P, KT, P).transpose(0, 3, 2, 1)
        ).reshape(MT, P, KT * P).astype(BF16_NP)
        w3p_np = np.ascontiguousarray(
            w3e.reshape(MT, P, KT, P).transpose(0, 3, 2, 1)
        ).reshape(MT, P, KT * P).astype(BF16_NP)
        # W2T[h, d] tiles: [hk, h, d]
        w2p_np = np.ascontiguousarray(w2e.T).reshape(MT, P, D).astype(BF16_NP)

        cfe = np.zeros(C, np.float32)
        cfe[:ne] = ce
        cf_np = np.ascontiguousarray(cfe.reshape(TT, P).T)

        in_maps.append(
            {"xt": xt_np, "w1p": w1p_np, "w3p": w3p_np, "w2p": w2p_np,
             "cf": cf_np}
        )

    # ---- build + run on 8 cores -------------------------------------------
    nc = build_kernel(C)
    res = None
    last_exc = None
    for attempt in range(3):
        try:
            res = run_bass_kernel_spmd(
                nc, in_maps, core_ids=list(range(E)),
                trace=TRACE and attempt == 0,
            )
            break
        except Exception as exc:  # transient device wedge / trace plumbing
            last_exc = exc
    if res is None:
        raise last_exc
    LAST_RESULTS = res

    # ---- combine ----------------------------------------------------------
    out = np.zeros((N, D), np.float32)
    for e in range(E):
        ide = idx_list[e]
        oe = res.results[e]["out"]  # [TT, 2, P, 512]
        oe = oe.transpose(0, 2, 1, 3).reshape(C, D)
        out[ide] += oe[: len(ide)]

    return out.reshape(B, S, D)



# revision 2
# speedup vs baseline: 1.0028x; 1.0028x over previous
"""MoE top-2 routed FFN (B=4, S=2048, D=1024, H=2048, E=8) on 8 TRN2 NeuronCores.

Strategy (expert-parallel with two-slot load balancing):
  - Host computes the tiny gate (softmax top-2) and folds each token's combine
    coefficient into its activation as x~ = c^(1/3) * x  (the expert FFN
    relu(xW1)^2*(xW3) @ W2 is degree-3 positively homogeneous in x, so the
    scaled input yields exactly c * FFN(x); padded tokens use c = 0).
  - Each core runs TWO fixed-size expert slots (A: CA tokens, B: CB tokens),
    each with its own weight set.  The busiest expert splits across two cores'
    A slots, the lightest across their B slots, the rest take one core's A+B.
    This drops per-core work from ceil128(max_e L_e) to ~avg_e L_e tokens.
  - Per slot: phase 1 computes gT[h, tok] = relu(W1 xT)^2 * (W3 xT) in bf16;
    phase 2 is "flipped" (W2T tiles stationary, token columns moving) so token
    counts need no 128-padding: psO[d, tok] accumulates over 16 h-tiles.
    Output [d, tok] tiles store bf16; host transposes and scatter-adds.
  - Engines: PE matmuls; DVE phase-1 elementwise; Scalar drains psO->SBUF and
    issues output DMAs; Sync issues all input DMAs (few, large transfers).
"""

import os
import sys

import numpy as np

if os.path.isdir("/opt/trn_rl_repo") and "/opt/trn_rl_repo" not in sys.path:
    sys.path.insert(0, "/opt/trn_rl_repo")

import ml_dtypes

import concourse.bacc as bacc
import concourse.mybir as mybir
from concourse.bass_utils import run_bass_kernel_spmd
from concourse.tile import TileContext

B, S, D, H, E = 4, 2048, 1024, 2048, 8
N = B * S
P = 128
KT = D // P   # 8 contraction tiles over D
MT = H // P   # 16 tiles over H
DT = D // P   # 8 output d-tiles (phase 2)

F32 = mybir.dt.float32
BF16 = mybir.dt.bfloat16
BF16_NP = ml_dtypes.bfloat16

# Set by test harness to capture profiling info.
TRACE = False
LAST_RESULTS = None


def _token_groups(c):
    """Split [0, c) into moving-dim groups of at most 512, min size 32."""
    groups = []
    off, rem = 0, c
    while rem > 0:
        g = min(512, rem)
        if 0 < rem - g < 32:
            g = rem - 32
        groups.append((off, g))
        off += g
        rem -= g
    return groups


def build_kernel(CA, CB):
    nc = bacc.Bacc("TRN2", target_bir_lowering=False)

    HA = min(512, CA)
    TA = CA - HA
    HB = min(512, CB)
    TB = CB - HB

    slots_meta = []
    for nm, C, HD, TL in (("a", CA, HA, TA), ("b", CB, HB, TB)):
        d = {
            "C": C, "H": HD, "T": TL,
            "xh": nc.dram_tensor(f"xh{nm}", [P, KT * HD], BF16,
                                 kind="ExternalInput"),
            "w13": nc.dram_tensor(f"w13{nm}", [MT, P, 2 * KT * P], BF16,
                                  kind="ExternalInput"),
            "w2": nc.dram_tensor(f"w2{nm}", [P, MT * D], BF16,
                                 kind="ExternalInput"),
            "out": nc.dram_tensor(f"out{nm}", [DT, P, C], BF16,
                                  kind="ExternalOutput"),
        }
        if TL > 0:
            d["xt"] = nc.dram_tensor(f"xt{nm}", [P, KT * TL], BF16,
                                     kind="ExternalInput")
        slots_meta.append(d)

    with TileContext(nc) as tc:
        with (
            tc.tile_pool(name="x_pool", bufs=2) as x_pool,
            tc.tile_pool(name="g_pool", bufs=1) as g_pool,
            tc.tile_pool(name="w13_pool", bufs=4) as w13_pool,
            tc.tile_pool(name="w2_pool", bufs=2) as w2_pool,
            tc.tile_pool(name="tmp_pool", bufs=2) as tmp_pool,
            tc.tile_pool(name="ob_pool", bufs=4) as ob_pool,
            tc.tile_pool(name="const_pool", bufs=1) as const_pool,
            tc.tile_pool(name="psAB", bufs=2, space="PSUM") as psAB_pool,
            tc.tile_pool(name="psO", bufs=4, space="PSUM") as psO_pool,
        ):
            # --- PE warmup: flip the HAM clock gate (1.2->2.4GHz) and keep
            # the PE busy until the first real operands land. ---------------
            warm = const_pool.tile([P, 512], BF16, tag="warm")
            nc.any.memset(warm[:], 0.0)
            pswarm = psO_pool.tile([P, 512], F32, tag="psO", name="pswarm")
            NWARM = 8
            for i in range(NWARM):
                nc.tensor.matmul(pswarm[:], warm[:, :P], warm[:],
                                 start=(i == 0), stop=(i == NWARM - 1))
            warmsink = const_pool.tile([P, 1], F32, tag="warmsink")
            nc.vector.tensor_scalar_mul(warmsink[:], pswarm[:, :1], 0.0)

            # per-slot runtime state
            st = [dict(), dict()]

            def emit_w13_dma(si, m):
                S = slots_meta[si]
                t = w13_pool.tile([P, 2 * KT * P], BF16, tag="w13",
                                  name=f"w13_{si}_{m}")
                nc.sync.dma_start(t[:], S["w13"][m])
                st[si].setdefault("w13", {})[m] = t

            def emit_w2_dma(si, q):
                S = slots_meta[si]
                if q == 0:
                    st[si]["w2"] = w2_pool.tile([P, MT * D], BF16, tag="w2",
                                                name=f"w2_{si}")
                w2t = st[si]["w2"]
                qw = MT * D // 4
                nc.sync.dma_start(w2t[:, q * qw:(q + 1) * qw],
                                  S["w2"][:, q * qw:(q + 1) * qw])

            def emit_slot_head_dmas(si):
                S = slots_meta[si]
                emit_w13_dma(si, 0)
                xh = x_pool.tile([P, KT * S["H"]], BF16, tag="xh",
                                 name=f"xh_{si}")
                nc.sync.dma_start(xh[:], S["xh"][:])
                st[si]["xh"] = xh
                emit_w13_dma(si, 1)
                if S["T"] > 0:
                    xt = x_pool.tile([P, KT * S["T"]], BF16, tag="xt",
                                     name=f"xt_{si}")
                    nc.sync.dma_start(xt[:], S["xt"][:])
                    st[si]["xt"] = xt

            def xt_slice(si, k, g0, gw):
                S = slots_meta[si]
                HD, TL = S["H"], S["T"]
                if g0 < HD:
                    assert g0 + gw <= HD
                    return st[si]["xh"][:, k * HD + g0:k * HD + g0 + gw]
                return st[si]["xt"][:, k * TL + g0 - HD:k * TL + g0 - HD + gw]

            def phase1_m(si, m):
                S = slots_meta[si]
                w13t = st[si]["w13"].pop(m)
                if m == 0:
                    gts = []
                    for j in range(MT):
                        gt = g_pool.tile([P, CA], BF16, tag=f"g{j}",
                                         name=f"g_{si}_{j}")
                        gts.append(gt)
                    st[si]["g"] = gts
                gt = st[si]["g"][m]
                for g0, gw in _token_groups(S["C"]):
                    psA = psAB_pool.tile([P, 512], F32, tag="psA",
                                         name=f"psA_{si}_{m}_{g0}")
                    psB = psAB_pool.tile([P, 512], F32, tag="psB",
                                         name=f"psB_{si}_{m}_{g0}")
                    for k in range(KT):
                        nc.tensor.matmul(
                            psA[:, :gw],
                            w13t[:, k * P:(k + 1) * P],
                            xt_slice(si, k, g0, gw),
                            start=(k == 0), stop=(k == KT - 1),
                        )
                    for k in range(KT):
                        nc.tensor.matmul(
                            psB[:, :gw],
                            w13t[:, KT * P + k * P:KT * P + (k + 1) * P],
                            xt_slice(si, k, g0, gw),
                            start=(k == 0), stop=(k == KT - 1),
                        )
                    r = tmp_pool.tile([P, 512], F32, tag="r",
                                      name=f"r_{si}_{m}_{g0}")
                    nc.vector.tensor_relu(r[:, :gw], psA[:, :gw])
                    t2 = tmp_pool.tile([P, 512], F32, tag="t2",
                                       name=f"t2_{si}_{m}_{g0}")
                    nc.vector.tensor_mul(t2[:, :gw], r[:, :gw], r[:, :gw])
                    nc.vector.tensor_mul(gt[:, g0:g0 + gw], t2[:, :gw],
                                         psB[:, :gw])

            def phase2_dt(si, dt):
                S = slots_meta[si]
                gts = st[si]["g"]
                w2t = st[si]["w2"]
                groups = _token_groups(S["C"])
                psOs = []
                for g0, gw in groups:
                    psOs.append(psO_pool.tile([P, 512], F32, tag="psO",
                                              name=f"psO_{si}_{dt}_{g0}"))
                for hk in range(MT):
                    wsl = w2t[:, hk * D + dt * P:hk * D + (dt + 1) * P]
                    for (g0, gw), ps in zip(groups, psOs):
                        nc.tensor.matmul(ps[:, :gw], wsl,
                                         gts[hk][:, g0:g0 + gw],
                                         start=(hk == 0), stop=(hk == MT - 1))
                for (g0, gw), ps in zip(groups, psOs):
                    ob = ob_pool.tile([P, 512], BF16, tag="ob",
                                      name=f"ob_{si}_{dt}_{g0}")
                    nc.scalar.copy(ob[:, :gw], ps[:, :gw])
                    nc.scalar.dma_start(S["out"][dt][:, g0:g0 + gw],
                                        ob[:, :gw])

            # ---- emission ------------------------------------------------
            emit_slot_head_dmas(0)
            W2Q_AT = (3, 6, 9, 12)
            for m in range(MT):
                if m + 2 < MT:
                    emit_w13_dma(0, m + 2)
                if m in W2Q_AT:
                    emit_w2_dma(0, W2Q_AT.index(m))
                phase1_m(0, m)

            emit_slot_head_dmas(1)
            for dt in range(DT):
                for j in (2 * dt + 2, 2 * dt + 3):
                    if j < MT:
                        emit_w13_dma(1, j)
                if dt in (1, 3, 5, 7):
                    emit_w2_dma(1, (1, 3, 5, 7).index(dt))
                phase2_dt(0, dt)

            for m in range(MT):
                phase1_m(1, m)
            for dt in range(DT):
                phase2_dt(1, dt)

    if not nc.is_finalized():
        nc.finalize()
    return nc


def _slot_plan(loads):
    """Two fixed slots (a >= b) per core; returns (a, b, assignment).

    assignment: list per core of dicts {"A": (expert, lo, hi), "B": ...}
    where [lo, hi) indexes into that expert's routed-token list.
    """
    L = np.asarray(loads)
    order = np.argsort(-L, kind="stable")
    hi, lo = order[0], order[-1]
    mids = order[1:-1]
    a = int(-(-L[hi] // 2))
    b = int(max(-(-L[lo] // 2), max(L[m] for m in mids) - a))
    asn = [None] * E
    asn[0] = {"A": (hi, 0, a), "B": (lo, 0, b)}
    asn[1] = {"A": (hi, a, int(L[hi])), "B": (lo, b, int(L[lo]))}
    for j, e in enumerate(mids):
        cut = min(a, int(L[e]))
        asn[2 + j] = {"A": (e, 0, cut), "B": (e, cut, int(L[e]))}
    # validate
    for c in asn:
        eA, l0, h0 = c["A"]
        eB, l1, h1 = c["B"]
        assert 0 <= h0 - l0 <= a and 0 <= h1 - l1 <= b
    return a, b, asn


def kernel(x, W1, W2, W3, gate_w, gate_b):
    global LAST_RESULTS

    xf = np.ascontiguousarray(x.reshape(N, D).astype(np.float32, copy=False))

    # ---- gate: softmax + top-2 (tiny, done on host) ------------------------
    logits = xf @ gate_w.T.astype(np.float32) + gate_b.astype(np.float32)
    logits -= logits.max(axis=-1, keepdims=True)
    probs = np.exp(logits)
    probs /= probs.sum(axis=-1, keepdims=True)
    order = np.argsort(-probs, axis=-1, kind="stable")
    i1, i2 = order[:, 0], order[:, 1]
    ar = np.arange(N)
    p1, p2 = probs[ar, i1], probs[ar, i2]
    ps = p1 + p2
    c1, c2 = p1 / ps, p2 / ps

    idx_list, coef_list = [], []
    for e in range(E):
        m1 = i1 == e
        m2 = i2 == e
        ide = np.nonzero(m1 | m2)[0]
        ce = np.where(m1[ide], c1[ide], c2[ide]).astype(np.float32)
        idx_list.append(ide)
        coef_list.append(ce)

    CA, CB, asn = _slot_plan([len(i) for i in idx_list])

    # ---- per-core input packing -------------------------------------------
    wpack_cache = {}

    def wpack(e):
        if e not in wpack_cache:
            w1e = np.asarray(W1[e], np.float32)
            w3e = np.asarray(W3[e], np.float32)
            w2e = np.asarray(W2[e], np.float32)
            w1p = w1e.reshape(MT, P, KT, P).transpose(0, 3, 2, 1)
            w3p = w3e.reshape(MT, P, KT, P).transpose(0, 3, 2, 1)
            w13 = np.ascontiguousarray(
                np.concatenate(
                    [w1p.reshape(MT, P, KT * P), w3p.reshape(MT, P, KT * P)],
                    axis=2)).astype(BF16_NP)
            w2p = np.ascontiguousarray(
                w2e.T.reshape(MT, P, D).transpose(1, 0, 2).reshape(P, MT * D)
            ).astype(BF16_NP)
            wpack_cache[e] = (w13, w2p)
        return wpack_cache[e]

    def xpack(e, l0, h0, C):
        ide = idx_list[e][l0:h0]
        c3 = np.cbrt(coef_list[e][l0:h0]).astype(np.float32)
        xg = np.zeros((C, D), np.float32)
        xg[:h0 - l0] = xf[ide] * c3[:, None]
        HD = min(512, C)
        T3 = np.ascontiguousarray(xg.T).reshape(KT, P, C)
        xh = np.ascontiguousarray(
            T3[:, :, :HD].transpose(1, 0, 2)).reshape(P, KT * HD)
        out = {"xh": xh.astype(BF16_NP)}
        if C > HD:
            xt = np.ascontiguousarray(
                T3[:, :, HD:].transpose(1, 0, 2)).reshape(P, KT * (C - HD))
            out["xt"] = xt.astype(BF16_NP)
        return out

    in_maps = []
    for c in range(E):
        m = {}
        for nm, C in (("a", CA), ("b", CB)):
            e, l0, h0 = asn[c]["A" if nm == "a" else "B"]
            w13, w2p = wpack(e)
            m[f"w13{nm}"] = w13
            m[f"w2{nm}"] = w2p
            xp = xpack(e, l0, h0, C)
            m[f"xh{nm}"] = xp["xh"]
            if "xt" in xp:
                m[f"xt{nm}"] = xp["xt"]
        in_maps.append(m)

    # ---- build + run on 8 cores -------------------------------------------
    nc = build_kernel(CA, CB)
    res = None
    last_exc = None
    for attempt in range(3):
        try:
            res = run_bass_kernel_spmd(
                nc, in_maps, core_ids=list(range(E)),
                trace=TRACE and attempt == 0,
            )
            break
        except Exception as exc:  # transient device wedge / trace plumbing
            last_exc = exc
    if res is None:
        raise last_exc
    LAST_RESULTS = res

    # ---- combine ----------------------------------------------------------
    out = np.zeros((N, D), np.float32)
    for c in range(E):
        for nm, C in (("a", CA), ("b", CB)):
            e, l0, h0 = asn[c]["A" if nm == "a" else "B"]
            if h0 <= l0:
                continue
            oe = res.results[c][f"out{nm}"].astype(np.float32)  # [DT, P, C]
            oe = oe.transpose(2, 0, 1).reshape(C, D)
            out[idx_list[e][l0:h0]] += oe[:h0 - l0]

    return out.reshape(B, S, D)


# revision 4
# speedup vs baseline: 1.0247x; 1.0219x over previous
"""MoE top-2 routed FFN (B=4, S=2048, D=1024, H=2048, E=8) on 8 TRN2 NeuronCores.

Strategy (expert-parallel with two-slot load balancing):
  - Host computes the tiny gate (softmax top-2) and folds each token's combine
    coefficient into its activation as x~ = c^(1/3) * x  (the expert FFN
    relu(xW1)^2*(xW3) @ W2 is degree-3 positively homogeneous in x, so the
    scaled input yields exactly c * FFN(x); padded tokens use c = 0).
  - Each core runs TWO fixed-size expert slots (A: CA tokens, B: CB tokens),
    each with its own weight set.  The busiest expert splits across two cores'
    A slots, the lightest across their B slots, the rest take one core's A+B.
    This drops per-core work from ceil128(max_e L_e) to ~avg_e L_e tokens.
  - Per slot: phase 1 computes gT[h, tok] = relu(W1 xT)^2 * (W3 xT) in bf16;
    phase 2 is "flipped" (W2T tiles stationary, token columns moving) so token
    counts need no 128-padding: psO[d, tok] accumulates over 16 h-tiles.
    Output [d, tok] tiles store bf16; host transposes and scatter-adds.
  - Engines: PE matmuls; DVE phase-1 elementwise; Scalar drains psO->SBUF and
    issues output DMAs; Sync issues weight DMAs; Vector/Scalar issue x DMAs.
"""

import os
import sys

import numpy as np

if os.path.isdir("/opt/trn_rl_repo") and "/opt/trn_rl_repo" not in sys.path:
    sys.path.insert(0, "/opt/trn_rl_repo")

import ml_dtypes

import concourse.bacc as bacc
import concourse.mybir as mybir
from concourse.bass_utils import run_bass_kernel_spmd
from concourse.tile import TileContext

B, S, D, H, E = 4, 2048, 1024, 2048, 8
N = B * S
P = 128
KT = D // P   # 8 contraction tiles over D
MT = H // P   # 16 tiles over H
DT = D // P   # 8 output d-tiles (phase 2)

F32 = mybir.dt.float32
BF16 = mybir.dt.bfloat16
BF16_NP = ml_dtypes.bfloat16

# Set by test harness to capture profiling info.
TRACE = False
LAST_RESULTS = None


def _token_groups(c):
    """Split [0, c) into moving-dim groups of at most 512, min size 32."""
    groups = []
    off, rem = 0, c
    while rem > 0:
        g = min(512, rem)
        if 0 < rem - g < 32:
            g = rem - 32
        groups.append((off, g))
        off += g
        rem -= g
    return groups


def build_kernel(CA, CB):
    nc = bacc.Bacc("TRN2", target_bir_lowering=False)

    HA = min(512, CA)
    TA = CA - HA
    HB = min(512, CB)
    TB = CB - HB

    slots_meta = []
    for nm, C, HD, TL in (("a", CA, HA, TA), ("b", CB, HB, TB)):
        d = {
            "C": C, "H": HD, "T": TL,
            "xt": nc.dram_tensor(f"xt{nm}", [KT, P, C], BF16,
                                 kind="ExternalInput"),
            "w13": nc.dram_tensor(f"w13{nm}", [MT, 2, P, KT * P], BF16,
                                  kind="ExternalInput"),
            "w2": nc.dram_tensor(f"w2{nm}", [MT, P, D], BF16,
                                 kind="ExternalInput"),
            "out": nc.dram_tensor(f"out{nm}", [DT, P, C], BF16,
                                  kind="ExternalOutput"),
        }
        slots_meta.append(d)

    with TileContext(nc) as tc:
        with (
            tc.tile_pool(name="x_pool", bufs=2) as x_pool,
            tc.tile_pool(name="g_pool", bufs=1) as g_pool,
            tc.tile_pool(name="w13_pool", bufs=4) as w13_pool,
            tc.tile_pool(name="w2_pool", bufs=2) as w2_pool,
            tc.tile_pool(name="tmp_pool", bufs=2) as tmp_pool,
            tc.tile_pool(name="ob_pool", bufs=4) as ob_pool,
            tc.tile_pool(name="const_pool", bufs=1) as const_pool,
            tc.tile_pool(name="psAB", bufs=2, space="PSUM") as psAB_pool,
            tc.tile_pool(name="psO", bufs=4, space="PSUM") as psO_pool,
        ):
            # --- PE warmup: flip the HAM clock gate (1.2->2.4GHz) and keep
            # the PE busy until the first real operands land. ---------------
            warm = const_pool.tile([P, 512], BF16, tag="warm")
            nc.any.memset(warm[:], 0.0)
            pswarm = psO_pool.tile([P, 512], F32, tag="psO", name="pswarm")
            NWARM = 8
            for i in range(NWARM):
                nc.tensor.matmul(pswarm[:], warm[:, :P], warm[:],
                                 start=(i == 0), stop=(i == NWARM - 1))
            warmsink = const_pool.tile([P, 1], F32, tag="warmsink")
            nc.vector.tensor_scalar_mul(warmsink[:], pswarm[:, :1], 0.0)

            # per-slot runtime state
            st = [dict(), dict()]

            def emit_w13_dma(si, m):
                S = slots_meta[si]
                w1t = w13_pool.tile([P, KT * P], BF16, tag="w1",
                                    name=f"w1_{si}_{m}")
                nc.sync.dma_start(w1t[:], S["w13"][m, 0])
                w3t = w13_pool.tile([P, KT * P], BF16, tag="w3",
                                    name=f"w3_{si}_{m}")
                nc.sync.dma_start(w3t[:], S["w13"][m, 1])
                st[si].setdefault("w13", {})[m] = (w1t, w3t)

            def emit_w2_dma(si, q):
                # quarter q: 4 of the 16 per-hk w2 tiles
                S = slots_meta[si]
                w2ts = st[si].setdefault("w2", [])
                for hk in range(4 * q, 4 * q + 4):
                    t = w2_pool.tile([P, D], BF16, tag=f"w2_{hk}",
                                     name=f"w2_{si}_{hk}")
                    nc.sync.dma_start(t[:], S["w2"][hk])
                    w2ts.append(t)

            def emit_x_dmas(si, part):
                S = slots_meta[si]
                HD, TL = S["H"], S["T"]
                if part == "h":
                    xhs = []
                    for k in range(KT):
                        xh = x_pool.tile([P, HD], BF16, tag=f"xh{k}",
                                         name=f"xh_{si}_{k}")
                        nc.gpsimd.dma_start(xh[:], S["xt"][k][:, :HD])
                        xhs.append(xh)
                    st[si]["xh"] = xhs
                elif TL > 0:
                    xts = []
                    for k in range(KT):
                        xt = x_pool.tile([P, TL], BF16, tag=f"xl{k}",
                                         name=f"xl_{si}_{k}")
                        nc.scalar.dma_start(xt[:], S["xt"][k][:, HD:])
                        xts.append(xt)
                    st[si]["xl"] = xts

            def xt_slice(si, k, g0, gw):
                S = slots_meta[si]
                HD = S["H"]
                if g0 < HD:
                    assert g0 + gw <= HD
                    return st[si]["xh"][k][:, g0:g0 + gw]
                return st[si]["xl"][k][:, g0 - HD:g0 - HD + gw]

            def phase1_m(si, m):
                S = slots_meta[si]
                w1t, w3t = st[si]["w13"].pop(m)
                if m == 0:
                    gts = []
                    for j in range(MT):
                        gt = g_pool.tile([P, CA], BF16, tag=f"g{j}",
                                         name=f"g_{si}_{j}")
                        gts.append(gt)
                    st[si]["g"] = gts
                gt = st[si]["g"][m]
                for g0, gw in _token_groups(S["C"]):
                    psA = psAB_pool.tile([P, 512], F32, tag="psA",
                                         name=f"psA_{si}_{m}_{g0}")
                    psB = psAB_pool.tile([P, 512], F32, tag="psB",
                                         name=f"psB_{si}_{m}_{g0}")
                    for k in range(KT):
                        nc.tensor.matmul(
                            psA[:, :gw],
                            w1t[:, k * P:(k + 1) * P],
                            xt_slice(si, k, g0, gw),
                            start=(k == 0), stop=(k == KT - 1),
                        )
                    for k in range(KT):
                        nc.tensor.matmul(
                            psB[:, :gw],
                            w3t[:, k * P:(k + 1) * P],
                            xt_slice(si, k, g0, gw),
                            start=(k == 0), stop=(k == KT - 1),
                        )
                    r = tmp_pool.tile([P, 512], F32, tag="r",
                                      name=f"r_{si}_{m}_{g0}")
                    nc.vector.tensor_relu(r[:, :gw], psA[:, :gw])
                    t2 = tmp_pool.tile([P, 512], F32, tag="t2",
                                       name=f"t2_{si}_{m}_{g0}")
                    nc.vector.tensor_mul(t2[:, :gw], r[:, :gw], r[:, :gw])
                    nc.vector.tensor_mul(gt[:, g0:g0 + gw], t2[:, :gw],
                                         psB[:, :gw])

            def phase2_dt(si, dt):
                S = slots_meta[si]
                gts = st[si]["g"]
                w2ts = st[si]["w2"]
                groups = _token_groups(S["C"])
                psOs = []
                for g0, gw in groups:
                    psOs.append(psO_pool.tile([P, 512], F32, tag="psO",
                                              name=f"psO_{si}_{dt}_{g0}"))
                for hk in range(MT):
                    wsl = w2ts[hk][:, dt * P:(dt + 1) * P]
                    for (g0, gw), ps in zip(groups, psOs):
                        nc.tensor.matmul(ps[:, :gw], wsl,
                                         gts[hk][:, g0:g0 + gw],
                                         start=(hk == 0), stop=(hk == MT - 1))
                for (g0, gw), ps in zip(groups, psOs):
                    ob = ob_pool.tile([P, 512], BF16, tag="ob",
                                      name=f"ob_{si}_{dt}_{g0}")
                    nc.scalar.copy(ob[:, :gw], ps[:, :gw])
                    nc.scalar.dma_start(S["out"][dt][:, g0:g0 + gw],
                                        ob[:, :gw])

            # ---- emission ------------------------------------------------
            emit_w13_dma(0, 0)
            emit_x_dmas(0, "h")
            emit_w13_dma(0, 1)
            emit_x_dmas(0, "t")
            W2Q_AT = (3, 6, 9, 12)
            for m in range(MT):
                if m + 2 < MT:
                    emit_w13_dma(0, m + 2)
                if m in W2Q_AT:
                    emit_w2_dma(0, W2Q_AT.index(m))
                phase1_m(0, m)

            emit_w13_dma(1, 0)
            emit_x_dmas(1, "h")
            emit_w13_dma(1, 1)
            emit_x_dmas(1, "t")
            for dt in range(DT):
                for j in (2 * dt + 2, 2 * dt + 3):
                    if j < MT:
                        emit_w13_dma(1, j)
                if dt in (1, 3, 5, 7):
                    emit_w2_dma(1, (1, 3, 5, 7).index(dt))
                phase2_dt(0, dt)

            for m in range(MT):
                phase1_m(1, m)
            for dt in range(DT):
                phase2_dt(1, dt)

    if not nc.is_finalized():
        nc.finalize()
    return nc


def _slot_plan(loads):
    """Two fixed slots (a >= b) per core; returns (a, b, assignment).

    assignment: list per core of dicts {"A": (expert, lo, hi), "B": ...}
    where [lo, hi) indexes into that expert's routed-token list.
    """
    L = np.asarray(loads)
    order = np.argsort(-L, kind="stable")
    hi, lo = order[0], order[-1]
    mids = order[1:-1]
    a = int(-(-L[hi] // 2))
    b = int(max(-(-L[lo] // 2), max(L[m] for m in mids) - a))
    asn = [None] * E
    asn[0] = {"A": (hi, 0, a), "B": (lo, 0, b)}
    asn[1] = {"A": (hi, a, int(L[hi])), "B": (lo, b, int(L[lo]))}
    for j, e in enumerate(mids):
        cut = min(a, int(L[e]))
        asn[2 + j] = {"A": (e, 0, cut), "B": (e, cut, int(L[e]))}
    # validate
    for c in asn:
        eA, l0, h0 = c["A"]
        eB, l1, h1 = c["B"]
        assert 0 <= h0 - l0 <= a and 0 <= h1 - l1 <= b
    return a, b, asn


def kernel(x, W1, W2, W3, gate_w, gate_b):
    global LAST_RESULTS

    xf = np.ascontiguousarray(x.reshape(N, D).astype(np.float32, copy=False))

    # ---- gate: softmax + top-2 (tiny, done on host) ------------------------
    logits = xf @ gate_w.T.astype(np.float32) + gate_b.astype(np.float32)
    logits -= logits.max(axis=-1, keepdims=True)
    probs = np.exp(logits)
    probs /= probs.sum(axis=-1, keepdims=True)
    order = np.argsort(-probs, axis=-1, kind="stable")
    i1, i2 = order[:, 0], order[:, 1]
    ar = np.arange(N)
    p1, p2 = probs[ar, i1], probs[ar, i2]
    ps = p1 + p2
    c1, c2 = p1 / ps, p2 / ps

    idx_list, coef_list = [], []
    for e in range(E):
        m1 = i1 == e
        m2 = i2 == e
        ide = np.nonzero(m1 | m2)[0]
        ce = np.where(m1[ide], c1[ide], c2[ide]).astype(np.float32)
        idx_list.append(ide)
        coef_list.append(ce)

    CA, CB, asn = _slot_plan([len(i) for i in idx_list])

    # ---- per-core input packing -------------------------------------------
    wpack_cache = {}

    def wpack(e):
        if e not in wpack_cache:
            w1e = np.asarray(W1[e], np.float32)
            w3e = np.asarray(W3[e], np.float32)
            w2e = np.asarray(W2[e], np.float32)
            w1p = w1e.reshape(MT, P, KT, P).transpose(0, 3, 2, 1)
            w3p = w3e.reshape(MT, P, KT, P).transpose(0, 3, 2, 1)
            w13 = np.ascontiguousarray(
                np.stack(
                    [w1p.reshape(MT, P, KT * P), w3p.reshape(MT, P, KT * P)],
                    axis=1)).astype(BF16_NP)
            w2p = np.ascontiguousarray(w2e.T).reshape(MT, P, D).astype(BF16_NP)
            wpack_cache[e] = (w13, w2p)
        return wpack_cache[e]

    def xpack(e, l0, h0, C):
        ide = idx_list[e][l0:h0]
        c3 = np.cbrt(coef_list[e][l0:h0]).astype(np.float32)
        xg = np.zeros((C, D), np.float32)
        xg[:h0 - l0] = xf[ide] * c3[:, None]
        return np.ascontiguousarray(xg.T).reshape(KT, P, C).astype(BF16_NP)

    in_maps = []
    for c in range(E):
        m = {}
        for nm, C in (("a", CA), ("b", CB)):
            e, l0, h0 = asn[c]["A" if nm == "a" else "B"]
            w13, w2p = wpack(e)
            m[f"w13{nm}"] = w13
            m[f"w2{nm}"] = w2p
            m[f"xt{nm}"] = xpack(e, l0, h0, C)
        in_maps.append(m)

    # ---- build + run on 8 cores -------------------------------------------
    nc = build_kernel(CA, CB)
    res = None
    last_exc = None
    for attempt in range(3):
        try:
            res = run_bass_kernel_spmd(
                nc, in_maps, core_ids=list(range(E)),
                trace=TRACE and attempt == 0,
            )
            break
        except Exception as exc:  # transient device wedge / trace plumbing
            last_exc = exc
    if res is None:
        raise last_exc
    LAST_RESULTS = res

    # ---- combine ----------------------------------------------------------
    out = np.zeros((N, D), np.float32)
    for c in range(E):
        for nm, C in (("a", CA), ("b", CB)):
            e, l0, h0 = asn[c]["A" if nm == "a" else "B"]
            if h0 <= l0:
                continue
            oe = res.results[c][f"out{nm}"].astype(np.float32)  # [DT, P, C]
            oe = oe.transpose(2, 0, 1).reshape(C, D)
            out[idx_list[e][l0:h0]] += oe[:h0 - l0]

    return out.reshape(B, S, D)
